# revision 8
# baseline (speedup 1.0000x reference)
"""Trainium2 Bass kernel for nn_GCM (GNN message passing / context GCN + FM decoder).

Strategy (8 NeuronCores, SPMD single NEFF):
  - Users/items/contexts/batch are range-sharded across cores.
  - Every segment_sum runs as destination-sorted one-hot matmuls accumulated in
    PSUM (gather -> DVE one-hot -> PE matmul), never scatter-add.
  - Edge lists are partitioned by destination shard on the host (index-only
    preprocessing); gathers use the dma_gather custom SWDGE instruction.
    SWDGE descriptor generation on the gpsimd engine (~8ns/row, serial) is the
    kernel bottleneck, so stage-0 feature aggregations over the SMALL tables
    (u_feat 4000 rows, c_feat 1000 rows) are instead computed densely on the
    PE: wide is_equal one-hot counts (DVE, int16 compare -> bf16), transposed
    via matmul-against-identity, then matmul against the bf16 feature table.
    PSUM->SBUF staging copies run on the idle Activation engine.
  - Full tables (encoded_u/i, ctx_mean, layer1_u/i) are replicated via
    AllGather collectives (shared-output RDH, ~300GB/s); layer-2 is computed
    only at the 4096 batch rows.  Collective launches are placed on the gpsimd
    queue right after their producer pass so they overlap the next pass.
  - FM decode runs fully on-chip; biases are fetched with indirect DMA.

The NEFF is identical on all cores; all per-core variation lives in the input
tensors.  Chunk/group structure is padded to the max over cores; padded edges
carry one-hot rank -1 (an all-zero one-hot column) so they contribute nothing.
"""

import numpy as np

import concourse.bacc as bacc
import concourse.bass as bass
import concourse.mybir as mybir
import concourse.tile as tile
from concourse import bass_utils
from concourse.library_config import mlp as _mlp_lib

# ---------------- problem constants (hardcoded; must match reference) -------
U, I, IT, C, E, D, B = 50000, 20000, 22000, 100000, 500000, 64, 4096
NCORES = 8
US, ISH, CS, BS = U // NCORES, I // NCORES, C // NCORES, B // NCORES
P = 128
NCHT = 48          # chunks per gather tile (multiple of OHB)
OHB = 8            # chunks per one-hot DVE op
SPLIT = 25000      # gather-table split size (int16 index headroom)
BLOCK = 49         # max dest groups per psum block (<= 56 psum slots)
NUF = 4096         # padded u_feat table rows (32 tiles)
NCF = 1024         # padded c_feat table rows (8 tiles)
F32 = mybir.dt.float32
BF16 = mybir.dt.bfloat16
I16 = mybir.dt.int16
I32 = mybir.dt.int32
ALU = mybir.AluOpType

G_U = (US + P - 1) // P      # 49
G_I = (ISH + P - 1) // P     # 20
G_C = (CS + P - 1) // P      # 98
G_B = (BS + P - 1) // P      # 4


def _dloc_cols(n_chunks):
    return (n_chunks // OHB + 2) * OHB


def _wrap_idx(v):
    """Pack an int index list (len % 128 == 0) into the [128, n/16] wrapped +
    8x-replicated int16 layout used by dma_gather."""
    n = len(v)
    a = v.reshape(n // 16, 16).T.astype(np.int16)
    return np.tile(a, (8, 1))


class PassPlan:
    """Core-uniform chunk schedule for one segment-sum pass."""

    def __init__(self, name, n_groups, n_sources):
        self.name = name
        self.n_groups = n_groups
        self.n_sources = n_sources
        self.chunk_group = []
        self.pieces = []           # (src, split, slot_lo, slot_hi, src_chunk_lo)
        self.blocks = []           # (g0, g1, slot_lo, slot_hi)
        self.n_chunks = 0
        self.runs = []             # (g, slot_lo, slot_hi)
        self.group_runs = {}
        self.group_last_run = {}

    def finish(self):
        self.n_chunks = len(self.chunk_group)
        for ri, (g, lo, hi) in enumerate(self.runs):
            self.group_runs[g] = self.group_runs.get(g, 0) + 1
            self.group_last_run[g] = ri


def _build_pass(name, n_groups, entries_per_core, splits, block=BLOCK):
    """Build a core-uniform segsum pass (see baseline docstring)."""
    nsrc = len(splits)
    ncor = len(entries_per_core)
    buckets = {}
    per_core_b = [dict() for _ in range(ncor)]
    for c in range(ncor):
        for s in range(nsrc):
            slot, idx = entries_per_core[c][s]
            sp = splits[s]
            h = idx // sp if sp else np.zeros_like(idx)
            g = slot // P
            key = (g.astype(np.int64) * 64 + s * 8 + h).astype(np.int64)
            order = np.argsort(key, kind="stable")
            ks, slot_s, idx_s, h_s = key[order], slot[order], idx[order], h[order]
            uk, starts = np.unique(ks, return_index=True)
            starts = list(starts) + [len(ks)]
            for j, k in enumerate(uk):
                gg, rem = divmod(int(k), 64)
                ss, hh = divmod(rem, 8)
                lo, hi = starts[j], starts[j + 1]
                per_core_b[c][(gg, ss, hh)] = (
                    slot_s[lo:hi] % P,
                    idx_s[lo:hi] - hh * (splits[s] or 0),
                )
                prev = buckets.get((gg, ss, hh), 0)
                buckets[(gg, ss, hh)] = max(prev, hi - lo)
    for g in range(n_groups):
        if not any(k[0] == g for k in buckets):
            buckets[(g, 0, 0)] = 1
    plan = PassPlan(name, n_groups, nsrc)
    src_chunks = [0] * nsrc
    gidx_parts = [[[] for _ in range(nsrc)] for _ in range(ncor)]
    dloc_parts = [[] for _ in range(ncor)]
    g0 = 0
    while g0 < n_groups:
        g1 = min(g0 + block, n_groups)
        slot_lo = len(plan.chunk_group)
        for s in range(nsrc):
            hs = sorted({k[2] for k in buckets if k[1] == s})
            for h in hs:
                run_lo = len(plan.chunk_group)
                for g in range(g0, g1):
                    n = buckets.get((g, s, h), 0)
                    if n == 0:
                        continue
                    nch = (n + P - 1) // P
                    npad = nch * P
                    plan.runs.append(
                        (g, len(plan.chunk_group), len(plan.chunk_group) + nch))
                    for c in range(ncor):
                        ent = per_core_b[c].get((g, s, h))
                        if ent is None:
                            ranks = np.full(npad, -1, np.int64)
                            idxs = np.zeros(npad, np.int64)
                        else:
                            r, ix = ent
                            pad = npad - len(r)
                            ranks = np.concatenate([r, np.full(pad, -1, np.int64)])
                            idxs = np.concatenate([ix, np.zeros(pad, np.int64)])
                        gidx_parts[c][s].append(idxs)
                        dloc_parts[c].append((len(plan.chunk_group), ranks))
                    plan.chunk_group.extend([g] * nch)
                run_hi = len(plan.chunk_group)
                if run_hi > run_lo:
                    plan.pieces.append((s, h, run_lo, run_hi, src_chunks[s]))
                    src_chunks[s] += run_hi - run_lo
        plan.blocks.append((g0, g1, slot_lo, len(plan.chunk_group)))
        g0 = g1
    plan.finish()
    plan.src_chunks = src_chunks
    ncol = _dloc_cols(plan.n_chunks)
    out = []
    for c in range(ncor):
        d = np.full((plan.n_chunks, P), -1.0, np.float32)
        for slot0, ranks in dloc_parts[c]:
            nch = len(ranks) // P
            d[slot0:slot0 + nch] = ranks.reshape(nch, P).astype(np.float32)
        dfull = np.full((P, ncol), -1.0, np.float32)
        dfull[:, :plan.n_chunks] = d.T
        arrs = {f"dloc_{name}": dfull}
        for s in range(nsrc):
            v = (np.concatenate(gidx_parts[c][s])
                 if gidx_parts[c][s] else np.zeros(0, np.int64))
            need = max(src_chunks[s], 1) * 8
            a = (_wrap_idx(v) if len(v)
                 else np.zeros((P, 8), np.int16))
            if a.shape[1] < need:
                a = np.concatenate(
                    [a, np.zeros((P, need - a.shape[1]), np.int16)], axis=1)
            arrs[f"gidx_{name}_{s}"] = a
        out.append(arrs)
    return plan, out


def _fields_arr(mat, n_rows, n_groups, pad_val):
    """[n_rows, 3] int field matrix -> [128, n_groups*3] int16 (col-major by
    group: column g*3+k holds field k of rows g*128..g*128+127)."""
    out = np.full((P, n_groups * 3), pad_val, np.int16)
    for g in range(n_groups):
        lo = g * P
        hi = min(lo + P, n_rows)
        for k in range(3):
            out[:hi - lo, g * 3 + k] = mat[lo:hi, k].astype(np.int16)
    return out


# ------------------------------------------------------------------------
def _host_prep(inputs):
    """Pure-integer host preprocessing: edge partitioning + pass plans."""
    u2 = np.asarray(inputs["insts2userid"])
    i2 = np.asarray(inputs["insts2itemid"])
    c2 = np.asarray(inputs["insts2contextid"])
    ufm = np.asarray(inputs["user_feature_mat"])
    ifm = np.asarray(inputs["item_feature_mat"])
    cfm = np.asarray(inputs["context_feature_mat"])
    uid = np.asarray(inputs["user_id"])
    iid = np.asarray(inputs["item_id"])
    cid = np.asarray(inputs["context_id"])

    plans = {}
    percore = [dict() for _ in range(NCORES)]

    def add(plan, arrs):
        plans[plan.name] = plan
        for c in range(NCORES):
            percore[c].update(arrs[c])

    # stage0 enci: 3 ifeat rows per item (kept as gathers: small + latency)
    ents = []
    for c in range(NCORES):
        sl = np.repeat(np.arange(ISH), 3)
        ix = ifm[c * ISH:(c + 1) * ISH].ravel().astype(np.int64)
        ents.append([(sl, ix)])
    add(*_build_pass("enci", G_I, ents, [None]))

    # stage0 ctx: item-source only (cfeat part is computed densely on PE)
    ents = []
    for c in range(NCORES):
        sh = cfm[c * CS:(c + 1) * CS]
        sl1 = np.arange(CS)
        ix1 = sh[:, 3].astype(np.int64)
        ents.append([(sl1, ix1)])
    add(*_build_pass("ctx", G_C, ents, [None]))

    # layer1 passes
    ents = []
    ucore = u2 // US
    for c in range(NCORES):
        m = ucore == c
        sl = (u2[m] % US).astype(np.int64)
        ents.append([(sl, i2[m].astype(np.int64)), (sl, c2[m].astype(np.int64))])
    add(*_build_pass("l1u", G_U, ents, [None, SPLIT]))

    ents = []
    icore = i2 // ISH
    for c in range(NCORES):
        m = icore == c
        sl = (i2[m] % ISH).astype(np.int64)
        ents.append([(sl, u2[m].astype(np.int64)), (sl, c2[m].astype(np.int64))])
    add(*_build_pass("l1i", G_I, ents, [SPLIT, SPLIT]))

    # CSRs for batch-restricted layer2
    uord = np.argsort(u2, kind="stable")
    ustart = np.searchsorted(u2[uord], np.arange(U + 1))
    iord = np.argsort(i2, kind="stable")
    istart = np.searchsorted(i2[iord], np.arange(I + 1))

    ents_u, ents_i = [], []
    for c in range(NCORES):
        bu = uid[c * BS:(c + 1) * BS]
        sl, ia, ca = [], [], []
        for b, u in enumerate(bu):
            ee = uord[ustart[u]:ustart[u + 1]]
            sl.append(np.full(len(ee), b, np.int64))
            ia.append(i2[ee].astype(np.int64))
            ca.append(c2[ee].astype(np.int64))
        sl = np.concatenate(sl) if sl else np.zeros(0, np.int64)
        ia = np.concatenate(ia) if ia else np.zeros(0, np.int64)
        ca = np.concatenate(ca) if ca else np.zeros(0, np.int64)
        ents_u.append([(sl, ia), (sl, ca)])

        bi = iid[c * BS:(c + 1) * BS]
        sl, ua, ca = [], [], []
        for b, it in enumerate(bi):
            ee = iord[istart[it]:istart[it + 1]]
            sl.append(np.full(len(ee), b, np.int64))
            ua.append(u2[ee].astype(np.int64))
            ca.append(c2[ee].astype(np.int64))
        sl = np.concatenate(sl) if sl else np.zeros(0, np.int64)
        ua = np.concatenate(ua) if ua else np.zeros(0, np.int64)
        ca = np.concatenate(ca) if ca else np.zeros(0, np.int64)
        ents_i.append([(sl, ua), (sl, ca)])
    add(*_build_pass("l2u", G_B, ents_u, [None, SPLIT]))
    add(*_build_pass("l2i", G_B, ents_i, [SPLIT, SPLIT]))

    # decode index arrays (512 per core)
    for c in range(NCORES):
        bu = uid[c * BS:(c + 1) * BS].astype(np.int64)
        bi = iid[c * BS:(c + 1) * BS].astype(np.int64)
        bc = cid[c * BS:(c + 1) * BS].astype(np.int64)
        cf = cfm[bc]
        pc = percore[c]
        m0 = (bu < SPLIT)
        pc["d_uh0"] = _wrap_idx(np.where(m0, bu, 0))
        pc["d_uh1"] = _wrap_idx(np.where(~m0, bu - SPLIT, 0))
        pc["d_umask0"] = m0.astype(np.float32).reshape(G_B, P).T.copy()
        pc["d_i"] = _wrap_idx(bi)
        for k in range(3):
            pc[f"d_cf{k}"] = _wrap_idx(cf[:, k].astype(np.int64))
        pc["d_cit"] = _wrap_idx(cf[:, 3].astype(np.int64))
        pc["d_ubias"] = bu.astype(np.int32).reshape(G_B, P).T.copy()
        pc["d_ibias"] = bi.astype(np.int32).reshape(G_B, P).T.copy()

    # per-core staging (padding / layout only, no float arithmetic)
    ue = np.asarray(inputs["user_emb"], np.float32)
    ie = np.asarray(inputs["item_emb"], np.float32)
    ufeat_pad = np.zeros((NUF, D), np.float32)
    ufeat_pad[:4000] = np.asarray(inputs["u_feat_emb"], np.float32)
    cfeat_pad = np.zeros((NCF, D), np.float32)
    cfeat_pad[:1000] = np.asarray(inputs["c_feat_emb"], np.float32)
    for c in range(NCORES):
        pc = percore[c]
        ub = np.zeros((G_U * P, D), np.float32)
        ub[:US] = ue[c * US:(c + 1) * US]
        pc["ue_base"] = ub
        ib = np.zeros((G_I * P, D), np.float32)
        ib[:ISH] = ie[c * ISH:(c + 1) * ISH]
        pc["ie_base"] = ib
        pc["t_ufeat"] = ufeat_pad
        pc["t_ifeat"] = np.asarray(inputs["i_feat_emb"], np.float32)
        pc["t_cfeat"] = cfeat_pad
        pc["t_item"] = ie
        pc["user_bias"] = np.asarray(inputs["user_bias"], np.float32)
        pc["item_bias"] = np.asarray(inputs["item_bias"], np.float32)
        pc["gbias"] = np.broadcast_to(
            np.asarray(inputs["global_bias"], np.float32).reshape(1, 1), (P, 1)
        ).copy()
        pc["f_encu"] = _fields_arr(ufm[c * US:(c + 1) * US], US, G_U, NUF - 1)
        pc["f_ctx"] = _fields_arr(cfm[c * CS:(c + 1) * CS, :3], CS, G_C,
                                  NCF - 1)
    return plans, percore


# ------------------------------------------------------------------------
def _emit_segsum(nc, pools, plan, src_aps, gidx_dram, dloc_dram, write_group,
                 acc, prefilled=False, tile_hook=None):
    """Emit one segsum pass (see baseline).  prefilled=True means acc already
    holds a partial sum (dense prefill), so the first run ADDS instead of
    copying.  tile_hook(i) is called after each gather tile's instructions."""
    gpool, ohpool, pspool, idxpool = (pools["g"], pools["oh"], pools["ps"],
                                      pools["idx"])
    dloc_t = pools["dloc"].tile([P, dloc_dram.shape[1]], F32, tag="dloc",
                                name="dloc")
    nc.sync.dma_start(out=dloc_t[:], in_=dloc_dram[:, :])
    iota_t = pools["iota"]

    seen_runs = {}
    open_ps = {}
    run_at = 0
    tile_no = 0
    for (g0, g1, slot_lo, slot_hi) in plan.blocks:
        for ts in range(slot_lo, slot_hi, NCHT):
            te = min(ts + NCHT, slot_hi)
            gt = gpool.tile([P, NCHT, D], F32, tag="gA", name="gt")
            for (spc, h, lo, hi, sclo) in plan.pieces:
                a, b = max(lo, ts), min(hi, te)
                if a >= b:
                    continue
                ca = sclo + (a - lo)
                nidx = (b - a) * P
                it = idxpool.tile([P, NCHT * 8], I16, tag="gi", name="gi")
                nc.sync.dma_start(
                    out=it[:, :(b - a) * 8],
                    in_=gidx_dram[spc][:, ca * 8:(ca + (b - a)) * 8],
                )
                nc.gpsimd.dma_gather(
                    gt[:, a - ts:b - ts, :], src_aps[spc][h],
                    it[:, :(b - a) * 8], nidx, nidx, D, single_packet=False,
                )
            ohs = []
            for ob in range(ts, te, OHB):
                oh = ohpool.tile([P, OHB * P], F32, tag="oh", name="oh")
                nc.vector.tensor_tensor(
                    out=oh[:].rearrange("p (c j) -> p c j", j=P),
                    in0=iota_t[:, :].unsqueeze(1).to_broadcast([P, OHB, P]),
                    in1=dloc_t[:, ob:ob + OHB].unsqueeze(2).to_broadcast(
                        [P, OHB, P]),
                    op=ALU.is_equal,
                )
                ohs.append(oh)
            for cslot in range(ts, te):
                while plan.runs[run_at][2] <= cslot:
                    run_at += 1
                g, rlo, rhi = plan.runs[run_at]
                if cslot == rlo:
                    open_ps[run_at] = pspool.tile([P, D], F32, tag="ps",
                                                  space="PSUM", name="ps")
                ps = open_ps[run_at]
                oh = ohs[(cslot - ts) // OHB]
                k = (cslot - ts) % OHB
                nc.tensor.matmul(
                    out=ps[:],
                    lhsT=oh[:, k * P:(k + 1) * P],
                    rhs=gt[:, cslot - ts, :],
                    start=(cslot == rlo),
                    stop=(cslot == rhi - 1),
                )
                if cslot == rhi - 1:
                    del open_ps[run_at]
                    nseen = seen_runs.get(g, 1 if prefilled else 0)
                    accsl = acc[:, g * D:(g + 1) * D]
                    if nseen == 0:
                        nc.vector.tensor_copy(out=accsl, in_=ps[:])
                    else:
                        nc.vector.tensor_tensor(out=accsl, in0=accsl,
                                                in1=ps[:], op=ALU.add)
                    seen_runs[g] = nseen + 1
                    if run_at == plan.group_last_run[g]:
                        write_group(g, accsl)
            if tile_hook is not None:
                tile_hook(tile_no)
            tile_no += 1


def build_kernel(plans):
    nc = bacc.Bacc("TRN2", target_bir_lowering=False, num_devices=NCORES)

    # ---- inputs
    def inp(name, shape, dt=F32):
        return nc.dram_tensor(name, shape, dt, kind="ExternalInput")

    t_ufeat = inp("t_ufeat", [NUF, D])
    t_ifeat = inp("t_ifeat", [4000, D])
    t_cfeat = inp("t_cfeat", [NCF, D])
    t_item = inp("t_item", [IT, D])
    ue_base = inp("ue_base", [G_U * P, D])
    ie_base = inp("ie_base", [G_I * P, D])
    user_bias = inp("user_bias", [U, 1])
    item_bias = inp("item_bias", [IT, 1])
    gbias = inp("gbias", [P, 1])
    f_encu = inp("f_encu", [P, G_U * 3], I16)
    f_ctx = inp("f_ctx", [P, G_C * 3], I16)

    gidx_in, dloc_in = {}, {}
    for name, plan in plans.items():
        dloc_in[name] = inp(f"dloc_{name}", [P, _dloc_cols(plan.n_chunks)])
        gidx_in[name] = [
            inp(f"gidx_{name}_{s}", [P, max(plan.src_chunks[s], 1) * 8], I16)
            for s in range(plan.n_sources)
        ]
    d_in = {}
    for nm in ("d_uh0", "d_uh1", "d_i", "d_cf0", "d_cf1", "d_cf2", "d_cit"):
        d_in[nm] = inp(nm, [P, BS // 16], I16)
    d_umask0 = inp("d_umask0", [P, G_B])
    d_ubias = inp("d_ubias", [P, G_B], I32)
    d_ibias = inp("d_ibias", [P, G_B], I32)

    out_b = nc.dram_tensor("out_b", [BS, 1], F32, kind="ExternalOutput")

    # ---- internal DRAM
    S_encu = nc.dram_tensor("S_encu", [US, D], F32)
    S_enci = nc.dram_tensor("S_enci", [ISH, D], F32)
    S_ctx = nc.dram_tensor("S_ctx", [CS, D], F32)
    S_l1u = nc.dram_tensor("S_l1u", [US, D], F32)
    S_l1i = nc.dram_tensor("S_l1i", [ISH, D], F32)
    T_encu = nc.dram_tensor("T_encu", [U, D], F32, addr_space="Shared")
    T_enci = nc.dram_tensor("T_enci", [I, D], F32, addr_space="Shared")
    T_ctx = nc.dram_tensor("T_ctx", [C, D], F32, addr_space="Shared")
    T_l1u = nc.dram_tensor("T_l1u", [U, D], F32, addr_space="Shared")
    T_l1i = nc.dram_tensor("T_l1i", [I, D], F32, addr_space="Shared")

    iota_np = np.tile(np.arange(P, dtype=np.float32), (P, 1))
    iota_dram = nc.inline_tensor(iota_np, name="iota128")
    iota16_np = np.tile(np.arange(NUF, dtype=np.int16), (P, 1))
    iota16_dram = nc.inline_tensor(iota16_np, name="iota16")
    ident_np = np.eye(P, dtype=np.float32)
    ident_dram = nc.inline_tensor(ident_np, name="ident128")

    RG = [list(range(NCORES))]

    with tile.TileContext(nc) as tc:
        with (
            tc.tile_pool(name="const", bufs=1) as constp,
            tc.tile_pool(name="g", bufs=3) as gpool,
            tc.tile_pool(name="oh", bufs=3) as ohpool,
            tc.tile_pool(name="idx", bufs=6) as idxpool,
            tc.tile_pool(name="dloc", bufs=2) as dlocp,
            tc.tile_pool(name="base", bufs=2) as basep,
            tc.tile_pool(name="acc", bufs=2) as accp,
            tc.tile_pool(name="tmp", bufs=6) as tmpp,
            tc.tile_pool(name="l2", bufs=1) as l2p,
            tc.tile_pool(name="actx", bufs=1) as actxp,
            tc.tile_pool(name="dec", bufs=1) as decp,
            tc.tile_pool(name="dm", bufs=1) as dmp,
            tc.tile_pool(name="lt", bufs=3) as ltp,
            tc.tile_pool(name="ps", bufs=4, space="PSUM") as pspool,
            tc.tile_pool(name="dps", bufs=2, space="PSUM") as dpspool,
        ):
            nc.gpsimd.load_library(_mlp_lib)
            iota_t = constp.tile([P, P], F32)
            nc.sync.dma_start(out=iota_t[:], in_=iota_dram[:, :])
            iota16_t = constp.tile([P, NUF], I16)
            nc.sync.dma_start(out=iota16_t[:], in_=iota16_dram[:, :])
            idf = constp.tile([P, P], F32)
            nc.sync.dma_start(out=idf[:], in_=ident_dram[:, :])
            ident_t = constp.tile([P, P], BF16)
            nc.scalar.copy(out=ident_t[:], in_=idf[:])
            fu_t = constp.tile([P, G_U * 3], I16)
            nc.sync.dma_start(out=fu_t[:], in_=f_encu[:, :])
            fc_t = constp.tile([P, G_C * 3], I16)
            nc.sync.dma_start(out=fc_t[:], in_=f_ctx[:, :])
            # bf16 copies of the dense feature tables, staged via SBUF
            uf_s = constp.tile([P, NUF // P, D], F32)
            nc.sync.dma_start(
                out=uf_s[:], in_=t_ufeat[:, :].rearrange("(t p) d -> p t d",
                                                         p=P))
            uf_b = constp.tile([P, NUF // P, D], BF16)
            nc.scalar.copy(out=uf_b[:], in_=uf_s[:])
            cf_s = constp.tile([P, NCF // P, D], F32)
            nc.sync.dma_start(
                out=cf_s[:], in_=t_cfeat[:, :].rearrange("(t p) d -> p t d",
                                                         p=P))
            cf_b = constp.tile([P, NCF // P, D], BF16)
            nc.scalar.copy(out=cf_b[:], in_=cf_s[:])

            pools = dict(g=gpool, oh=ohpool, ps=pspool, idx=idxpool,
                         dloc=dlocp, iota=iota_t)

            def seg(name, src_aps, write_group, acc=None, prefilled=False,
                    tile_hook=None):
                if acc is None:
                    acc = accp.tile([P, plans[name].n_groups * D], F32,
                                    tag="acc", name="acc")
                _emit_segsum(nc, pools, plans[name], src_aps,
                             gidx_in[name], dloc_in[name], write_group, acc,
                             prefilled=prefilled, tile_hook=tile_hook)
                return acc

            def store_shard(S, g, sl, scale=None, base_t=None):
                rows = min(P, S.shape[0] - g * P)
                t = tmpp.tile([P, D], F32, tag="cp", name="cp")
                if scale is not None:
                    nc.vector.tensor_scalar(out=t[:], in0=sl, scalar1=scale,
                                            scalar2=None, op0=ALU.mult)
                else:
                    nc.vector.tensor_copy(out=t[:], in_=sl)
                if base_t is not None:
                    nc.vector.tensor_tensor(
                        out=t[:], in0=t[:],
                        in1=base_t[:, g * D:(g + 1) * D], op=ALU.add)
                nc.sync.dma_start(out=S[g * P:g * P + rows, :], in_=t[:rows, :])

            def dense_group(fields_t, tbl_b, n_tiles, g):
                """PSUM [P, D] accum of sum_k table[field_k] for group g."""
                width = n_tiles * P
                M = dmp.tile([P, width], BF16, tag="M", name="M")
                Mk = dmp.tile([P, width], BF16, tag="Mk", name="Mk")
                for k in range(3):
                    dst = M if k == 0 else Mk
                    nc.vector.tensor_tensor(
                        out=dst[:],
                        in0=fields_t[:, g * 3 + k:g * 3 + k + 1].to_broadcast(
                            [P, width]),
                        in1=iota16_t[:, :width], op=ALU.is_equal)
                    if k:
                        nc.vector.tensor_tensor(out=M[:], in0=M[:], in1=Mk[:],
                                                op=ALU.add)
                accd = dpspool.tile([P, D], F32, tag="dacc", space="PSUM",
                                    name="dacc")
                for t in range(n_tiles):
                    tp = dpspool.tile([P, P], F32, tag="tp", space="PSUM",
                                      name="tp")
                    nc.tensor.matmul(out=tp[:], lhsT=M[:, t * P:(t + 1) * P],
                                     rhs=ident_t[:], start=True, stop=True)
                    lt = ltp.tile([P, P], BF16, tag="lt", name="lt")
                    nc.scalar.copy(out=lt[:], in_=tp[:])
                    nc.tensor.matmul(out=accd[:], lhsT=lt[:],
                                     rhs=tbl_b[:, t, :], start=(t == 0),
                                     stop=(t == n_tiles - 1))
                return accd

            # ---------- stage0: enci (gathers) ----------
            ib_t = basep.tile([P, G_I * D], F32, tag="base")
            nc.sync.dma_start(
                out=ib_t[:].rearrange("p (g d) -> p g d", d=D),
                in_=ie_base[:, :].rearrange("(g p) d -> p g d", p=P))
            nc.vector.tensor_scalar(out=ib_t[:], in0=ib_t[:], scalar1=0.25,
                                    scalar2=None, op0=ALU.mult)
            seg("enci", [[t_ifeat[:, :]]],
                lambda g, sl: store_shard(S_enci, g, sl, 0.25, ib_t))
            nc.gpsimd.collective_compute(
                "AllGather", ALU.bypass, RG,
                ins=[S_enci[:, :].opt()], outs=[T_enci[:, :].opt()])

            # ---------- stage0: ctx (dense cfeat prefill + item gathers) ----
            acc_ctx = actxp.tile([P, G_C * D], F32, tag="accctx",
                                 name="acc_ctx")
            # a group's dense prefill must be emitted before its first
            # run-end add in the item seg: derive the requirement per tile
            ctx_plan = plans["ctx"]
            first_end = {}
            for (g, rlo, rhi) in ctx_plan.runs:
                first_end[g] = min(first_end.get(g, 1 << 30), rhi)
            tile_bounds = []
            for (_, _, slot_lo, slot_hi) in ctx_plan.blocks:
                for ts in range(slot_lo, slot_hi, NCHT):
                    tile_bounds.append((ts, min(ts + NCHT, slot_hi)))
            need_by_tile = {}
            for g, fe in first_end.items():
                t_i = next(j for j, (ts, te) in enumerate(tile_bounds)
                           if fe - 1 < te) - 1
                need_by_tile[t_i] = max(need_by_tile.get(t_i, 0), g + 1)
            ctx_state = {"g": 0}

            def ctx_emit(n):
                while ctx_state["g"] < min(G_C, n):
                    g = ctx_state["g"]
                    accd = dense_group(fc_t, cf_b, NCF // P, g)
                    nc.scalar.copy(out=acc_ctx[:, g * D:(g + 1) * D],
                                   in_=accd[:])
                    ctx_state["g"] += 1

            ctx_emit(need_by_tile.get(-1, 0) + 4)

            def ctx_hook(i):
                ctx_emit(max(need_by_tile.get(i, 0),
                             ctx_state["g"] + 4))

            seg("ctx", [[t_item[:, :]]],
                lambda g, sl: store_shard(S_ctx, g, sl, scale=0.25),
                acc=acc_ctx, prefilled=True, tile_hook=ctx_hook)
            ctx_emit(G_C)

            # decode raw-table gathers early: gpsimd filler during AGs
            def dgather(table_ap, idx_dram, tag):
                it = idxpool.tile([P, BS // 16], I16, tag="gi", name="gi")
                nc.sync.dma_start(out=it[:], in_=idx_dram[:, :])
                t = decp.tile([P, G_B, D], F32, tag=tag, name=tag)
                nc.gpsimd.dma_gather(t[:], table_ap, it[:], BS, BS, D)
                return t

            cfd = [dgather(t_cfeat[:, :], d_in[f"d_cf{k}"], f"cf{k}")
                   for k in range(3)]
            cit = dgather(t_item[:, :], d_in["d_cit"], "cit")

            nc.gpsimd.collective_compute(
                "AllGather", ALU.bypass, RG,
                ins=[S_ctx[:, :].opt()], outs=[T_ctx[:, :].opt()])

            # ---------- stage0: encu (dense; overlapped with l1u via hook) ---
            ub_t = basep.tile([P, G_U * D], F32, tag="base")
            nc.sync.dma_start(
                out=ub_t[:].rearrange("p (g d) -> p g d", d=D),
                in_=ue_base[:, :].rearrange("(g p) d -> p g d", p=P))
            nc.vector.tensor_scalar(out=ub_t[:], in0=ub_t[:], scalar1=0.25,
                                    scalar2=None, op0=ALU.mult)
            encu_state = {"g": 0}

            def encu_emit(n):
                while encu_state["g"] < min(G_U, n):
                    g = encu_state["g"]
                    accd = dense_group(fu_t, uf_b, NUF // P, g)
                    store_shard(S_encu, g, accd[:], scale=0.25, base_t=ub_t)
                    encu_state["g"] += 1

            encu_emit(4)

            def l1u_hook(i):
                encu_emit(4 + (i + 1) * 3)

            # ---------- layer 1 ----------
            ctx_splits = [T_ctx[h * SPLIT:(h + 1) * SPLIT, :] for h in range(4)]
            seg("l1u", [[T_enci[:, :]], ctx_splits],
                lambda g, sl: store_shard(S_l1u, g, sl), tile_hook=l1u_hook)
            encu_emit(G_U)
            nc.gpsimd.collective_compute(
                "AllGather", ALU.bypass, RG,
                ins=[S_encu[:, :].opt()], outs=[T_encu[:, :].opt()])
            nc.gpsimd.collective_compute(
                "AllGather", ALU.bypass, RG,
                ins=[S_l1u[:, :].opt()], outs=[T_l1u[:, :].opt()])

            encu_splits = [T_encu[h * SPLIT:(h + 1) * SPLIT, :]
                           for h in range(2)]
            seg("l1i", [encu_splits, ctx_splits],
                lambda g, sl: store_shard(S_l1i, g, sl))
            nc.gpsimd.collective_compute(
                "AllGather", ALU.bypass, RG,
                ins=[S_l1i[:, :].opt()], outs=[T_l1i[:, :].opt()])

            # ---------- layer 2 (batch-restricted, stays in SBUF) ----------
            l1u_splits = [T_l1u[h * SPLIT:(h + 1) * SPLIT, :]
                          for h in range(2)]
            acc_l2i = l2p.tile([P, G_B * D], F32, tag="l2i")
            seg("l2i", [l1u_splits, ctx_splits],
                lambda g, sl: None, acc=acc_l2i)
            acc_l2u = l2p.tile([P, G_B * D], F32, tag="l2u")
            seg("l2u", [[T_l1i[:, :]], ctx_splits],
                lambda g, sl: None, acc=acc_l2u)

            # ---------- decode ----------
            def tt(out, a, b, op):
                nc.vector.tensor_tensor(out=out, in0=a, in1=b, op=op)

            m0 = decp.tile([P, G_B], F32, tag="m0")
            nc.sync.dma_start(out=m0[:], in_=d_umask0[:, :])
            m0b = m0[:, :].unsqueeze(2).to_broadcast([P, G_B, D])

            eu0 = dgather(T_encu[0:SPLIT, :], d_in["d_uh0"], "eu0")
            eu1 = dgather(T_encu[SPLIT:U, :], d_in["d_uh1"], "eu1")
            lu0 = dgather(T_l1u[0:SPLIT, :], d_in["d_uh0"], "lu0")
            lu1 = dgather(T_l1u[SPLIT:U, :], d_in["d_uh1"], "lu1")
            tt(eu0[:], eu0[:], lu0[:], ALU.add)
            tt(eu1[:], eu1[:], lu1[:], ALU.add)
            tt(eu0[:], eu0[:], eu1[:], ALU.subtract)
            tt(eu0[:], eu0[:], m0b, ALU.mult)
            fin_u = decp.tile([P, G_B, D], F32, tag="finu")
            tt(fin_u[:], eu0[:], eu1[:], ALU.add)
            l2u_v = acc_l2u[:].rearrange("p (g d) -> p g d", d=D)
            tt(fin_u[:], fin_u[:], l2u_v, ALU.add)
            nc.vector.tensor_scalar(out=fin_u[:], in0=fin_u[:],
                                    scalar1=1.0 / 3.0, scalar2=None,
                                    op0=ALU.mult)

            ei = dgather(T_enci[:, :], d_in["d_i"], "ei")
            li = dgather(T_l1i[:, :], d_in["d_i"], "li")
            fin_i = decp.tile([P, G_B, D], F32, tag="fini")
            tt(fin_i[:], ei[:], li[:], ALU.add)
            l2i_v = acc_l2i[:].rearrange("p (g d) -> p g d", d=D)
            tt(fin_i[:], fin_i[:], l2i_v, ALU.add)
            nc.vector.tensor_scalar(out=fin_i[:], in0=fin_i[:],
                                    scalar1=1.0 / 3.0, scalar2=None,
                                    op0=ALU.mult)

            rows = [fin_u, fin_i, cfd[0], cfd[1], cfd[2], cit]
            S = decp.tile([P, G_B, D], F32, tag="S")
            tt(S[:], rows[0][:], rows[1][:], ALU.add)
            for r in rows[2:]:
                tt(S[:], S[:], r[:], ALU.add)
            SS = decp.tile([P, G_B, D], F32, tag="SS")
            tt(SS[:], S[:], S[:], ALU.mult)
            Q = decp.tile([P, G_B, D], F32, tag="Q")
            tt(Q[:], rows[0][:], rows[0][:], ALU.mult)
            sq = decp.tile([P, G_B, D], F32, tag="sq")
            for r in rows[1:]:
                tt(sq[:], r[:], r[:], ALU.mult)
                tt(Q[:], Q[:], sq[:], ALU.add)
            tt(SS[:], SS[:], Q[:], ALU.subtract)
            red = decp.tile([P, G_B], F32, tag="red")
            nc.vector.tensor_reduce(out=red[:].unsqueeze(2), in_=SS[:],
                                    axis=mybir.AxisListType.X, op=ALU.add)
            nc.vector.tensor_scalar(out=red[:], in0=red[:], scalar1=0.5,
                                    scalar2=None, op0=ALU.mult)

            ub_i = decp.tile([P, G_B], I32, tag="ubi")
            ib_i = decp.tile([P, G_B], I32, tag="ibi")
            nc.sync.dma_start(out=ub_i[:], in_=d_ubias[:, :])
            nc.sync.dma_start(out=ib_i[:], in_=d_ibias[:, :])
            bu = decp.tile([P, G_B], F32, tag="bu")
            bi = decp.tile([P, G_B], F32, tag="bi")
            for j in range(G_B):
                nc.gpsimd.indirect_dma_start(
                    out=bu[:, j:j + 1], out_offset=None, in_=user_bias[:, :],
                    in_offset=bass.IndirectOffsetOnAxis(ap=ub_i[:, j:j + 1],
                                                        axis=0))
                nc.gpsimd.indirect_dma_start(
                    out=bi[:, j:j + 1], out_offset=None, in_=item_bias[:, :],
                    in_offset=bass.IndirectOffsetOnAxis(ap=ib_i[:, j:j + 1],
                                                        axis=0))
            gb_t = decp.tile([P, 1], F32, tag="gb")
            nc.sync.dma_start(out=gb_t[:], in_=gbias[:, :])
            tt(red[:], red[:], bu[:], ALU.add)
            tt(red[:], red[:], bi[:], ALU.add)
            nc.vector.tensor_scalar(out=red[:], in0=red[:],
                                    scalar1=gb_t[:, :1], scalar2=None,
                                    op0=ALU.add)
            nc.sync.dma_start(
                out=out_b[:, :].rearrange("(g p) d -> p g d", p=P),
                in_=red[:].unsqueeze(2))

    nc.compile()
    return nc


def kernel(**inputs):
    plans, percore = _host_prep(inputs)
    nc = build_kernel(plans)
    res = bass_utils.run_bass_kernel_spmd(
        nc, percore, core_ids=list(range(NCORES)))
    out = np.concatenate([res.results[c]["out_b"].reshape(-1)
                          for c in range(NCORES)])
    return out.astype(np.float32)
